# revision 16
# baseline (speedup 1.0000x reference)
"""AChebyKANLinear forward on 8 TRN2 NeuronCores (data-parallel over batch).

y = silu(x) @ W_base^T + einsum('bid,iod->bo', cos(n_d * arccos(tanh x)), gated_coeffs)

Key identities used:
  cos(n*arccos(c)) = T_n(c)  (Chebyshev), c = tanh(x)
  -> no trig needed on device. Device computes 13 "columns" per feature:
     silu(x), and 12 cheap polynomials of c whose exact Chebyshev-basis
     expansion is tracked symbolically on the host; the host solves a small
     linear system to fold the change of basis into the matmul weights.
  Even T_2m come from ACT Square ops (T_2m+1 = 2*T_m^2), odd ones from single
  fused DVE scalar_tensor_tensor ops. All columns bf16; one big
  [4096, 3328] @ [3328, 256] GEMM per core on TensorE (fp32 PSUM accum).

Top-k routing over the 8 logits is computed on the host (it is 8 numbers);
the 4 selected high degrees are baked into the compiled graph.
"""

import numpy as np
import ml_dtypes
from contextlib import ExitStack

import concourse.bass as bass
import concourse.tile as tile
from concourse import bacc, mybir
from concourse.bass_utils import run_bass_kernel_spmd

BF16 = ml_dtypes.bfloat16

N_CORES = 8
BATCH, I_DIM, O_DIM = 32768, 256, 256
B_LOC = BATCH // N_CORES          # 4096
# graduated batch chunks: small first chunks shorten the pipeline fill before
# TensorE has all 13 columns of chunk 0; steady state runs at 1024.
CHUNK_SIZES = [256, 256] + [512] * 7
assert sum(CHUNK_SIZES) == B_LOC
BC_MAX = max(CHUNK_SIZES)
DEGREE = 16
BASE_DEGREES = 8
TOPK = 4

SQ2 = float(np.sqrt(2.0))

A = mybir.ActivationFunctionType
ALU = mybir.AluOpType
F32 = mybir.dt.float32
DBF16 = mybir.dt.bfloat16


# ---------------- symbolic Chebyshev algebra (host, exact) ----------------

def _chmul(a, b):
    out = np.zeros(40)
    nz_a = np.nonzero(a)[0]
    nz_b = np.nonzero(b)[0]
    for i in nz_a:
        for j in nz_b:
            p = a[i] * b[j] * 0.5
            out[i + j] += p
            out[abs(i - j)] += p
    return out


def _e(n):
    v = np.zeros(40)
    v[n] = 1.0
    return v


def _recipe(S):
    """Build the per-chunk op recipe and each column's Chebyshev expansion.

    Returns (ops, colvec) where ops is a list of
      ('act', name, src, func, scale, bias) or
      ('stt', name, in0, scalar, op0, in1, op1) or
      ('tt',  name, in0, in1, op)
    and colvec maps tile name -> length-40 Chebyshev coefficient vector.
    """
    ops = []
    vec = {}

    def act(name, src, func, scale=1.0, bias=0.0):
        ops.append(("act", name, src, func, float(scale), float(bias)))
        if func == A.Square:
            aff = vec[src] * scale
            aff[0] += bias
            vec[name] = _chmul(aff, aff)
        elif func == A.Tanh:
            vec[name] = _e(1)
        else:  # Silu: not a Chebyshev column
            vec[name] = None

    def stt(name, in0, scalar, op0, in1, op1):
        ops.append(("stt", name, in0, float(scalar), op0, in1, op1))
        a = vec[in0].copy()
        if op0 == ALU.add:
            a[0] += scalar
        elif op0 == ALU.mult:
            a = a * scalar
        else:
            raise ValueError(op0)
        b = vec[in1]
        if op1 == ALU.mult:
            vec[name] = _chmul(a, b)
        elif op1 == ALU.subtract:
            vec[name] = a - b
        elif op1 == ALU.add:
            vec[name] = a + b
        else:
            raise ValueError(op1)

    def tt(name, in0, in1, op):
        ops.append(("tt", name, in0, in1, op))
        if op == ALU.subtract:
            vec[name] = vec[in0] - vec[in1]
        elif op == ALU.add:
            vec[name] = vec[in0] + vec[in1]
        elif op == ALU.mult:
            vec[name] = _chmul(vec[in0], vec[in1])
        else:
            raise ValueError(op)

    act("silu", "x", A.Silu)
    act("c1", "x", A.Tanh)
    act("c2", "c1", A.Square, SQ2)            # T2 + 1
    act("c4", "c2", A.Square, SQ2, -SQ2)      # T4 + 1
    act("c8", "c4", A.Square, SQ2, -SQ2)      # T8 + 1
    stt("c3", "c2", -1.5, ALU.add, "c1", ALU.mult)   # (c2-1.5)*c1 = T3/2
    stt("c5", "c4", -1.0, ALU.add, "c1", ALU.mult)   # T4*T1 = (T5+T3)/2
    stt("c6", "c3", 2.0, ALU.mult, "c3", ALU.mult)   # 2*c3^2 = (T6+1)/2  (DVE)
    stt("c7", "c4", -1.0, ALU.add, "c3", ALU.mult)   # T4*T3/2 = (T7+T1)/4
    for n in S:
        if n == 9:
            stt("c9", "c8", -1.0, ALU.add, "c1", ALU.mult)    # T8*T1
        elif n == 10:
            act("c10", "c5", A.Square, SQ2)                    # 2*c5^2
        elif n == 11:
            stt("c11", "c8", -1.0, ALU.add, "c3", ALU.mult)   # T8*T3/2
        elif n == 12:
            act("c12", "c6", A.Square, 2.0 * SQ2, -SQ2)        # 2*(2*c6-1)^2 = T12+1
        elif n == 13:
            tt("d53", "c5", "c3", ALU.subtract)                # T5/2
            stt("c13", "c8", -1.0, ALU.add, "d53", ALU.mult)  # T8*T5/2
        elif n == 14:
            stt("c14", "c8", -1.0, ALU.add, "c6", ALU.mult)   # T8*(T6+1)/2
        elif n == 15:
            stt("t7p", "c7", 4.0, ALU.mult, "c1", ALU.subtract)  # T7
            stt("c15", "c8", -1.0, ALU.add, "t7p", ALU.mult)     # T8*T7
        elif n == 16:
            act("c16", "c8", A.Square, SQ2, -SQ2)              # T16+1
        else:
            raise ValueError(n)
    return ops, vec


def _solve_basis(S, low_degrees):
    """Solve for X s.t. sum_col X[col,n]*colvec[col] == e_n for each needed n.

    Columns: 'bias' (the constant 1) + the 12 device Chebyshev columns.
    Returns (ops, matmul_cols, X) with X keyed [col][n].
    """
    ops, vec = _recipe(S)
    cheb_cols = ["c1", "c2", "c3", "c4", "c5", "c6", "c7", "c8"] + [f"c{n}" for n in S]
    needed = sorted(set(int(n) for n in low_degrees) | set(S))
    Amat = np.zeros((40, 1 + len(cheb_cols)))
    Amat[0, 0] = 1.0  # bias column = T_0
    for j, cn in enumerate(cheb_cols):
        Amat[:, 1 + j] = vec[cn]
    X = {}
    for n in needed:
        sol, res, rank, _ = np.linalg.lstsq(Amat, _e(n), rcond=None)
        err = np.abs(Amat @ sol - _e(n)).max()
        assert err < 1e-9, f"basis solve failed for degree {n}: {err}"
        X[n] = sol  # [1+len(cheb_cols)]
    return ops, cheb_cols, X


# ---------------- device graph ----------------

def _build_nc(S, niter=1):
    ops, cheb_cols, _ = _solve_basis(S, range(BASE_DEGREES + 1))
    blocks = ["silu"] + cheb_cols          # 13 matmul K-blocks
    n_kk = 2 * len(blocks)                 # 26 k-tiles of 128

    nc = bacc.Bacc("TRN2", target_bir_lowering=False, debug=False,
                   num_devices=N_CORES)
    # register const APs for the activation bias values we use (only 0.0/1.0
    # are pre-registered); mirrors Bass.__init__'s register_const_ap.
    bias_consts = sorted({op[5] for op in ops if op[0] == "act"} - {0.0})
    for v in bias_consts:
        t_c = nc.alloc_sbuf_tensor(f"const-f32-{v}", [128, 1], F32)
        nc.gpsimd.memset(t_c.ap(), v)
        nc.const_aps.aps[(F32, v)] = t_c.ap()
    if bias_consts:
        nc.all_engine_barrier()
    x_d = nc.dram_tensor("xt", [128, 2 * B_LOC], F32, kind="ExternalInput").ap()
    w_d = nc.dram_tensor("w", [128, n_kk * O_DIM], DBF16, kind="ExternalInput").ap()
    b_d = nc.dram_tensor("bias", [O_DIM, 1], F32, kind="ExternalInput").ap()
    o_d = nc.dram_tensor("out", [O_DIM, B_LOC], F32, kind="ExternalOutput").ap()

    with tile.TileContext(nc) as tc, ExitStack() as ctx:
        cpool = ctx.enter_context(tc.tile_pool(name="const", bufs=1))
        xpool = ctx.enter_context(tc.tile_pool(name="x", bufs=3))
        rpool = ctx.enter_context(tc.tile_pool(name="r", bufs=4))
        tpool = ctx.enter_context(tc.tile_pool(name="tmp", bufs=3))
        opool = ctx.enter_context(tc.tile_pool(name="o", bufs=8))
        pspool = ctx.enter_context(tc.tile_pool(name="ps", bufs=8, space="PSUM"))

        wt = cpool.tile([128, n_kk * O_DIM], DBF16)
        nc.sync.dma_start(wt[:], w_d[:])
        bt = []
        for m in range(2):
            b_tile = cpool.tile([128, 1], F32, tag=f"bias{m}")
            nc.sync.dma_start(b_tile[:], b_d[m * 128:(m + 1) * 128, :])
            bt.append(b_tile)

        chunks = []
        off = 0
        for bc in CHUNK_SIZES:
            chunks.append((off, bc))
            off += bc
        for it in range(niter):
            for ci, (off, bc) in enumerate(chunks):
                cc = f"{it}_{ci}"
                xt = xpool.tile([128, 2 * bc], F32, tag="xt", name=f"xt{cc}")
                nc.sync.dma_start(xt[:], x_d[:, 2 * off: 2 * (off + bc)])

                tiles = {"x": xt}
                for op in ops:
                    kind, name = op[0], op[1]
                    pool = rpool if (name in blocks) else tpool
                    t = pool.tile([128, 2 * bc], DBF16, tag=name,
                                  name=f"{name}_{cc}")
                    if kind == "act":
                        _, _, src, func, scale, bias_v = op
                        nc.scalar.activation(t[:], tiles[src][:], func,
                                             bias=bias_v, scale=scale)
                    elif kind == "stt":
                        _, _, in0, scalar, op0, in1, op1 = op
                        nc.vector.scalar_tensor_tensor(t[:], tiles[in0][:], scalar,
                                                       tiles[in1][:], op0, op1)
                    else:  # tt
                        _, _, in0, in1, alu = op
                        nc.vector.tensor_tensor(t[:], tiles[in0][:],
                                                tiles[in1][:], alu)
                    tiles[name] = t

                nsubs = [(s, min(512, bc - s)) for s in range(0, bc, 512)]
                for m in range(2):
                    for so, sn in nsubs:
                        ps = pspool.tile([128, sn], F32, tag="ps",
                                         name=f"ps{cc}_{m}_{so}")
                        for kk in range(n_kk):
                            j, h = kk // 2, kk % 2
                            rt = tiles[blocks[j]]
                            nc.tensor.matmul(
                                ps[:],
                                wt[:, kk * O_DIM + m * 128: kk * O_DIM + (m + 1) * 128],
                                rt[:, h * bc + so: h * bc + so + sn],
                                start=(kk == 0), stop=(kk == n_kk - 1),
                            )
                        ot = opool.tile([128, sn], F32, tag="ot",
                                        name=f"ot{cc}_{m}_{so}")
                        nc.vector.tensor_scalar(ot[:], ps[:], bt[m][:], None, ALU.add)
                        nc.sync.dma_start(
                            o_d[m * 128:(m + 1) * 128, off + so: off + so + sn],
                            ot[:])

    nc.compile()
    return nc


_NC_CACHE = {}


def _get_nc(S, niter=1):
    key = (tuple(S), niter)
    if key not in _NC_CACHE:
        _NC_CACHE[key] = _build_nc(S, niter)
    return _NC_CACHE[key]


# ---------------- host wrapper ----------------

def _prepare(x, logits, cheby_coeffs, base_weight, gating_weights, arange):
    x = np.asarray(x, dtype=np.float32)
    logits = np.asarray(logits, dtype=np.float32)
    cheby_coeffs = np.asarray(cheby_coeffs, dtype=np.float32)
    base_weight = np.asarray(base_weight, dtype=np.float32)
    gating_weights = np.asarray(gating_weights, dtype=np.float32)
    arange = np.asarray(arange)

    # top-k routing (host; 8 numbers). Matches jax.lax.top_k ordering.
    order = np.argsort(-logits, kind="stable")[:TOPK]
    topk_vals = 1.0 / (1.0 + np.exp(-logits[order].astype(np.float64)))
    gate = gating_weights.astype(np.float64).copy()
    sel = order + BASE_DEGREES + 1
    gate[sel] = topk_vals
    S = sorted(int(v) for v in sel)

    low = sorted(int(v) for v in arange)   # normally [0..8]
    ops, cheb_cols, X = _solve_basis(S, low)

    # weight blocks in K order: ['silu'] + cheb_cols
    G = {n: gate[n] * cheby_coeffs[:, :, n].astype(np.float64) for n in set(low) | set(S)}
    blocks = [base_weight.T.astype(np.float64)]
    bias = np.zeros(O_DIM, dtype=np.float64)
    for j, cn in enumerate(cheb_cols):
        Wb = np.zeros((I_DIM, O_DIM), dtype=np.float64)
        for n, sol in X.items():
            coef = sol[1 + j]
            if coef != 0.0 and n in G:
                Wb += coef * G[n]
        blocks.append(Wb)
    for n, sol in X.items():
        if sol[0] != 0.0 and n in G:
            bias += sol[0] * G[n].sum(axis=0)

    # interleave into device layout: W_sb[p, kk*O + o], kk = 2*j + h, i = 128*h + p
    n_blocks = len(blocks)
    Wsb = np.empty((128, 2 * n_blocks, O_DIM), dtype=np.float32)
    for j, Wb in enumerate(blocks):
        Wf = Wb.astype(np.float32)
        Wsb[:, 2 * j + 0, :] = Wf[0:128, :]
        Wsb[:, 2 * j + 1, :] = Wf[128:256, :]
    w_np = Wsb.reshape(128, 2 * n_blocks * O_DIM).astype(BF16)
    bias_np = bias.astype(np.float32).reshape(O_DIM, 1)
    return S, w_np, bias_np


def _make_xt(xl):
    """xt[p, 2*off + h*bc + bb] = xl[off+bb, 128*h+p] for each chunk (off, bc)."""
    out = np.empty((128, 2 * B_LOC), dtype=np.float32)
    off = 0
    for bc in CHUNK_SIZES:
        blk = xl[off:off + bc, :].reshape(bc, 2, 128).transpose(2, 1, 0)
        out[:, 2 * off: 2 * (off + bc)] = blk.reshape(128, 2 * bc)
        off += bc
    return out


def kernel(x, t, logits, cheby_coeffs, base_weight, gating_weights, arange):
    x = np.asarray(x, dtype=np.float32)
    S, w_np, bias_np = _prepare(x, logits, cheby_coeffs, base_weight,
                                gating_weights, arange)
    nc = _get_nc(S)

    in_maps = []
    for c in range(N_CORES):
        xt = _make_xt(x[c * B_LOC:(c + 1) * B_LOC, :])
        in_maps.append({"xt": xt, "w": w_np, "bias": bias_np})

    res = run_bass_kernel_spmd(nc, in_maps, core_ids=list(range(N_CORES)))
    y = np.empty((BATCH, O_DIM), dtype=np.float32)
    for c in range(N_CORES):
        y[c * B_LOC:(c + 1) * B_LOC, :] = res.results[c]["out"].T
    return y
